# revision 47
# baseline (speedup 1.0000x reference)
"""GCN (3-layer message passing + mean-pool + MLP head) on 8 Trainium2 NeuronCores.

Sharding: nodes and their incident (by dst) edges are sharded across 8 cores;
per layer the dinv-scaled features are AllGathered into bf16 gather tables in
DRAM (three region tensors A/B/C so the last, boundary-exposed collective is
tiny), each core dma_gathers its edges' src rows (4 SWDGE queues, round-robin)
and segment-sums them with indicator matmuls on the TensorEngine. Indicator
tiles are prebuilt on the host (dinv[dst] folded in) and streamed from DRAM;
deg/dinv, the pooling one-hot (1/cnt folded) and x^T are host-side structure
prep. bg enters via a reserved table row + an all-ones indicator slot.
Pooling is an indicator matmul plus one [128,256] AllReduce; the MLP head is
computed redundantly.
"""

import numpy as np
import ml_dtypes
from dataclasses import dataclass, field

BF16 = ml_dtypes.bfloat16


# ---------------------------------------------------------------------------
# Config
# ---------------------------------------------------------------------------
@dataclass
class Cfg:
    N: int = 50000          # nodes
    E: int = 600000         # edges
    F: int = 128            # feature dim
    NL: int = 3             # gcn layers
    G: int = 256            # graphs
    H: int = 256            # hidden dim of head
    LD: int = 2             # label dim
    C: int = 8              # cores
    WA: int = 25            # windows in table region A
    WB: int = 21            # windows in table region B (C region = rest)
    PF: int = 6             # gather prefetch distance (windows)
    PFI: int = 3            # indicator-DMA prefetch distance

    @property
    def NPC(self):          # nodes per core
        return self.N // self.C

    @property
    def W(self):            # 128-node windows per core
        return (self.NPC + 127) // 128

    @property
    def NPAD(self):         # padded nodes per core
        return self.W * 128

    @property
    def RA(self):           # local rows in region A
        return 128 * self.WA

    @property
    def RB(self):           # local rows in region B
        return 128 * self.WB

    @property
    def RC(self):           # local rows in region C
        return self.NPAD - self.RA - self.RB


@dataclass
class Meta:
    """Uniform (core-independent) graph structure + per-core data arrays."""
    m: list = field(default_factory=list)        # [3][W] subchunk counts
    in_maps: list = field(default_factory=list)  # per-core tensor dicts


# ---------------------------------------------------------------------------
# Host-side sharding / layout prep (structure only, no weight math)
# ---------------------------------------------------------------------------
def _wrap16(arr_i16):
    # slot i -> [i % 16, i // 16]; 16-row wrap replicated to 128 partitions
    # (one copy per GPSIMD Q7 core).
    return np.ascontiguousarray(np.tile(arr_i16.reshape(-1, 16).T, (8, 1)))


def host_prep(cfg: Cfg, x, Wg, bg, w1, b1, w2, b2, edge_index, batch) -> Meta:
    C, NPC, W, NPAD, F, G = cfg.C, cfg.NPC, cfg.W, cfg.NPAD, cfg.F, cfg.G
    RA, RB, RC = cfg.RA, cfg.RB, cfg.RC
    src = np.asarray(edge_index[0], dtype=np.int64)
    dst = np.asarray(edge_index[1], dtype=np.int64)
    batch = np.asarray(batch, dtype=np.int64)
    x = np.asarray(x, dtype=np.float32)

    # GCN norm (self-loops included): structure-only prep
    indeg = np.bincount(dst, minlength=cfg.N)
    deg = indeg.astype(np.float64) + 1.0
    dinv = (1.0 / np.sqrt(deg)).astype(np.float32)

    # degree-balanced node -> (core, local slot) assignment (LPT): equalizes
    # per-(core, window) edge counts so the uniform max-over-core subchunk
    # counts carry less padding. Window W-1 keeps the NPC..NPAD pad slots.
    import heapq
    nbins = C * W
    cap = np.full(nbins, 128, np.int64)
    cap[W - 1::W] = NPC - 128 * (W - 1)          # last window short
    heap = [(0, int(b)) for b in range(nbins)]
    heapq.heapify(heap)
    fill = np.zeros(nbins, np.int64)
    bin_nodes = [[] for _ in range(nbins)]
    order_nodes = np.argsort(-indeg, kind="stable")
    for g_ in order_nodes:
        while True:
            wgt, b = heapq.heappop(heap)
            if fill[b] < cap[b]:
                break
        bin_nodes[b].append(g_)
        fill[b] += 1
        if fill[b] < cap[b]:
            heapq.heappush(heap, (wgt + int(indeg[g_]), b))
    slot_core = np.empty(cfg.N, np.int64)
    slot_sl = np.empty(cfg.N, np.int64)
    core_nodes = []                               # per core: node id per slot
    for c in range(C):
        ns = []
        for w in range(W):
            ns.extend(bin_nodes[c * W + w])
        ns = np.asarray(ns, np.int64)
        assert len(ns) == NPC
        core_nodes.append(ns)
        slot_core[ns] = c
        slot_sl[ns] = np.arange(NPC)

    # stream of a src node and its region-relative table row
    sc, sl = slot_core[src], slot_sl[src]
    region = np.where(sl < RA, 0, np.where(sl < RA + RB, 1, 2))
    rrow = np.where(sl < RA, sc * RA + sl,
                    np.where(sl < RA + RB, sc * RB + (sl - RA),
                             sc * RC + (sl - RA - RB)))
    J_BG = NPC - RA - RB          # core-0 pad row 6250, C-region-relative

    # per (core, window, region) edge lists
    per_core = []
    dst_core, dst_sl = slot_core[dst], slot_sl[dst]
    for c in range(C):
        m = dst_core == c
        d_c, r_c, t_c = dst[m], region[m], rrow[m]
        dloc = dst_sl[m]
        order = np.argsort(dloc, kind="stable")
        dloc, r_c, t_c = dloc[order], r_c[order], t_c[order]
        dv = dinv[d_c[order]]
        win = dloc // 128
        drel = dloc - win * 128
        lists = []
        for w in range(W):
            wm = win == w
            lists.append(tuple(
                (t_c[wm & (r_c == s)], drel[wm & (r_c == s)],
                 dv[wm & (r_c == s)])
                for s in range(3)))
        per_core.append(lists)

    # uniform subchunk counts (max over cores); region C carries one extra
    # bg slot per window (slot 0 of the window's first C subchunk)
    mm = [[0] * W for _ in range(3)]
    for w in range(W):
        for s in range(3):
            extra = 1 if s == 2 else 0
            need = max(len(per_core[c][w][s][0]) + extra for c in range(C))
            mm[s][w] = max(1, -(-need // 128)) if s == 2 else -(-need // 128)
    L = [128 * sum(mm[s]) for s in range(3)]
    S = sum(sum(mm[s]) for s in range(3))
    # SWDGE desc ring holds ~128 descriptors (n/16+1 per gather): keep every
    # call well below that or the Q7 await_space deadlocks on hardware
    assert all(128 * max(mm[s]) <= 1920 for s in range(3)), mm

    meta = Meta(m=mm)

    # global graph-level pooling weights (1/cnt folded into the one-hot)
    cnt = np.bincount(batch, minlength=G).astype(np.float32)
    inv_cnt = 1.0 / np.maximum(cnt, 1.0)

    # per-window subchunk offsets (window-major: A then B then C subchunks)
    gsub0 = []
    o = 0
    for w in range(W):
        gsub0.append(o)
        o += mm[0][w] + mm[1][w] + mm[2][w]
    assert o == S

    wgb = np.concatenate([np.asarray(Wg[l], np.float32) for l in range(cfg.NL)],
                         axis=1).astype(BF16)                       # [F, NL*F]
    bgrow = np.asarray(bg, np.float32).reshape(1, cfg.NL * F).astype(BF16)
    identB = np.eye(128, dtype=np.float32).astype(BF16)

    for c in range(C):
        idxs = [np.zeros(L[s], np.int16) for s in range(3)]
        # indicator blocks [S, 128slot, 128dst] f32
        ind = np.zeros((S, 128, 128), np.float32)
        offs = [0, 0, 0]
        for w in range(W):
            s0 = gsub0[w]
            for s in range(3):
                tl, dl, vl = per_core[c][w][s]
                n = len(tl)
                o = offs[s]
                if s == 2:
                    idxs[s][o] = np.int16(J_BG)
                    idxs[s][o + 1:o + 1 + n] = tl.astype(np.int16)
                    ind[s0, 0, :] = 1.0
                    sub = (1 + np.arange(n)) // 128
                    ind[s0 + sub, (1 + np.arange(n)) % 128, dl] = vl
                else:
                    idxs[s][o:o + n] = tl.astype(np.int16)
                    sub = np.arange(n) // 128
                    ind[s0 + sub, np.arange(n) % 128, dl] = vl
                offs[s] += 128 * mm[s][w]
                s0 += mm[s][w]
        assert offs == L
        indT = np.ascontiguousarray(
            ind.transpose(1, 0, 2).reshape(128, S * 128)).astype(BF16)

        nodes_c = core_nodes[c]
        # x^T resident [F, NPAD] bf16
        xs = np.zeros((NPAD, F), np.float32)
        xs[:NPC] = x[nodes_c]
        xsT = np.ascontiguousarray(xs.T).astype(BF16)

        # dinv per local node [128, W], pads 0
        dv = np.zeros(NPAD, np.float32)
        dv[:NPC] = dinv[nodes_c]
        dinvS = np.ascontiguousarray(dv.reshape(W, 128).T)

        # per-window diag(dinv) for the self-loop matmul [128, W*128]
        diag = np.zeros((W, 128, 128), np.float32)
        dvw = dv.reshape(W, 128)
        diag[:, np.arange(128), np.arange(128)] = dvw
        diagS = np.ascontiguousarray(
            diag.transpose(1, 0, 2).reshape(128, W * 128)).astype(BF16)

        # pooling one-hot with 1/cnt folded [128, W*G]
        ig = np.zeros((NPAD, G), np.float32)
        bloc = batch[nodes_c]
        ig[np.arange(NPC), bloc] = inv_cnt[bloc]
        igS = np.ascontiguousarray(
            ig.reshape(W, 128, G).transpose(1, 0, 2).reshape(128, W * G)
        ).astype(BF16)

        b2B = np.tile(np.asarray(b2, np.float32).reshape(1, cfg.LD), (128, 1))

        meta.in_maps.append(dict(
            xsT=xsT,
            srcA=_wrap16(idxs[0]),
            srcB=_wrap16(idxs[1]),
            srcC=_wrap16(idxs[2]),
            ind=indT,
            igS=igS,
            diagS=diagS,
            dinvS=dinvS,
            wgb=wgb,
            bgrow=bgrow,
            identB=identB,
            w1=np.asarray(w1, np.float32),
            b1=np.asarray(b1, np.float32).reshape(cfg.H, 1),
            w2=np.asarray(w2, np.float32),
            b2B=b2B,
        ))
    return meta


# ---------------------------------------------------------------------------
# Device graph
# ---------------------------------------------------------------------------
def build_graph(cfg: Cfg, meta: Meta):
    import concourse.bass as bass
    import concourse.bacc as bacc
    import concourse.mybir as mybir
    import concourse.tile as tile

    F, W, NL, NPAD = cfg.F, cfg.W, cfg.NL, cfg.NPAD
    GR = cfg.G
    f32, bf16, i16 = mybir.dt.float32, mybir.dt.bfloat16, mybir.dt.int16
    AL = mybir.AluOpType
    ACT = mybir.ActivationFunctionType
    mm = meta.m
    L = [128 * sum(mm[s]) for s in range(3)]
    S = sum(sum(mm[s]) for s in range(3))
    # SWDGE desc ring holds ~128 descriptors (n/16+1 per gather): keep every
    # call well below that or the Q7 await_space deadlocks on hardware
    assert all(128 * max(mm[s]) <= 1920 for s in range(3)), mm
    Mmax = [max(mm[s]) for s in range(3)]
    MImax = max(mm[0][w] + mm[1][w] + mm[2][w] for w in range(W))
    REG = [cfg.RA, cfg.RB, cfg.RC]
    RE0 = [0, cfg.RA, cfg.RA + cfg.RB]          # local row offsets
    WEND = [cfg.WA - 1, cfg.WA + cfg.WB - 1, W - 1]  # emit windows per region

    # per-window offsets
    offw = [[0] * W for _ in range(3)]
    offS = [0] * W
    acc = [0, 0, 0]
    s_acc = 0
    for w in range(W):
        offS[w] = s_acc
        for s in range(3):
            offw[s][w] = acc[s]
            acc[s] += 128 * mm[s][w]
            s_acc += mm[s][w]

    nc = bacc.Bacc("TRN2", target_bir_lowering=False, debug=False,
                   num_devices=cfg.C, num_swdge_queues=4)

    # --- external IO ------------------------------------------------------
    P = {}
    P["xsT"] = nc.declare_dram_parameter("xsT", [F, NPAD], bf16, isOutput=False)
    for s, nm in enumerate(("srcA", "srcB", "srcC")):
        P[nm] = nc.declare_dram_parameter(nm, [128, L[s] // 16], i16,
                                          isOutput=False)
    P["ind"] = nc.declare_dram_parameter("ind", [128, S * 128], bf16, isOutput=False)
    P["igS"] = nc.declare_dram_parameter("igS", [128, W * GR], bf16, isOutput=False)
    P["diagS"] = nc.declare_dram_parameter("diagS", [128, W * 128], bf16, isOutput=False)
    P["dinvS"] = nc.declare_dram_parameter("dinvS", [128, W], f32, isOutput=False)
    P["wgb"] = nc.declare_dram_parameter("wgb", [F, NL * F], bf16, isOutput=False)
    P["bgrow"] = nc.declare_dram_parameter("bgrow", [1, NL * F], bf16, isOutput=False)
    P["identB"] = nc.declare_dram_parameter("identB", [128, 128], bf16, isOutput=False)
    P["w1"] = nc.declare_dram_parameter("w1", [F, cfg.H], f32, isOutput=False)
    P["b1"] = nc.declare_dram_parameter("b1", [cfg.H, 1], f32, isOutput=False)
    P["w2"] = nc.declare_dram_parameter("w2", [cfg.H, cfg.LD], f32, isOutput=False)
    P["b2B"] = nc.declare_dram_parameter("b2B", [128, cfg.LD], f32, isOutput=False)
    out_ext = nc.declare_dram_parameter("out", [GR, cfg.LD], f32, isOutput=True)

    # --- internal DRAM ----------------------------------------------------
    tables = [[nc.dram_tensor(f"table{s}{i}", [cfg.C * REG[s], F], bf16,
                              addr_space="Shared") for i in range(2)]
              for s in range(3)]
    shards = [[nc.dram_tensor(f"shard{s}{i}", [REG[s], F], bf16)
               for i in range(2)] for s in range(3)]
    arInD = nc.dram_tensor("arInD", [128, GR], f32)
    arOutD = nc.dram_tensor("arOutD", [128, GR], f32, addr_space="Shared")

    rg = [list(range(cfg.C))]

    with tile.TileContext(nc) as tc:
        with (
            tc.tile_pool(name="res", bufs=1) as res,      # resident tensors
            tc.tile_pool(name="work", bufs=5) as work,    # per-window temps
            tc.tile_pool(name="indp", bufs=5) as indp,    # indicator stream
            tc.tile_pool(name="g0", bufs=13) as gp0,       # region-A gathers
            tc.tile_pool(name="g1", bufs=13) as gp1,       # region-B gathers
            tc.tile_pool(name="g2", bufs=9) as gp2,       # region-C gathers
            tc.tile_pool(name="ps", bufs=2, space="PSUM") as ps,
            tc.tile_pool(name="ps3", bufs=4, space="PSUM") as ps3,
            tc.tile_pool(name="ps1", bufs=1, space="PSUM") as ps1,
            tc.tile_pool(name="psacc", bufs=1, space="PSUM") as psacc,
        ):
            gpools = [gp0, gp1, gp2]
            # ---------------- resident loads ----------------
            wgS = res.tile([128, NL * F], bf16, tag="wgS")
            nc.sync.dma_start(wgS[:], P["wgb"][:])
            dinvS = res.tile([128, W], f32, tag="dinvS")
            nc.sync.dma_start(dinvS[:], P["dinvS"][:])
            xsT = res.tile([128, NPAD], bf16, tag="xsT")
            nc.sync.dma_start(xsT[:, 0:NPAD // 2], P["xsT"][:, 0:NPAD // 2])
            srcS = []
            for s, nm in enumerate(("srcA", "srcB", "srcC")):
                t = res.tile([128, L[s] // 16], i16, tag=nm)
                nc.sync.dma_start(t[:], P[nm][:])
                srcS.append(t)
            nc.sync.dma_start(xsT[:, NPAD // 2:], P["xsT"][:, NPAD // 2:])
            ident = res.tile([128, 128], bf16, tag="ident")
            nc.sync.dma_start(ident[:], P["identB"][:])
            w1S = res.tile([128, cfg.H], f32, tag="w1S")
            nc.sync.dma_start(w1S[:], P["w1"][:])
            w2S = res.tile([128, 2 * cfg.LD], f32, tag="w2S")
            nc.sync.dma_start(w2S[:, 0:cfg.LD], P["w2"][0:128, :])
            nc.sync.dma_start(w2S[:, cfg.LD:2 * cfg.LD], P["w2"][128:256, :])
            b1S = res.tile([128, 2], f32, tag="b1S")
            nc.sync.dma_start(b1S[:, 0:1], P["b1"][0:128, :])
            nc.sync.dma_start(b1S[:, 1:2], P["b1"][128:256, :])
            b2B = res.tile([128, cfg.LD], f32, tag="b2B")
            nc.sync.dma_start(b2B[:], P["b2B"][:])
            diagS = res.tile([128, W * 128], bf16, tag="diagS")
            nc.sync.dma_start(diagS[:], P["diagS"][:])

            hbuf = res.tile([128, W * F], bf16, tag="hbuf")      # h' chunks, node-major

            # init gather pools (stale-SBUF NaN guard for padded tails)
            for s, nb in ((0, 13), (1, 13), (2, 9)):
                for _ in range(nb):
                    t = gpools[s].tile([128, Mmax[s] * F], bf16, tag=f"g{s}")
                    nc.vector.memset(t[:], 0.0)

            def emit_shard(s, par):
                shardD, tableD = shards[s][par], tables[s][par]
                r0 = RE0[s]
                nc.sync.dma_start(
                    shardD[:].rearrange("(w p) f -> p w f", p=128),
                    hbuf[:, r0 * F // 128:(r0 + REG[s]) * F // 128]
                    .rearrange("p (w f) -> p w f", f=F))
                nc.gpsimd.collective_compute(
                    "AllGather", mybir.AluOpType.bypass, replica_groups=rg,
                    ins=[shardD[:]], outs=[tableD[:]])

            def poke_bg(l):
                # write bg_l into hbuf pad row (node 6250: window 48, p=106)
                pw = cfg.NPC // 128
                pp = cfg.NPC - pw * 128
                nc.sync.dma_start(hbuf[pp:pp + 1, pw * F:(pw + 1) * F],
                                  P["bgrow"][:, l * F:(l + 1) * F])

            # ---------------- phase A: h'_0 = dinv * (x @ Wg0) -----------
            for w in range(W):
                hP = ps.tile([128, F], f32, tag="hP")
                nc.tensor.matmul(hP[:], xsT[:, w * 128:(w + 1) * 128],
                                 wgS[:, 0:F], start=True, stop=True)
                nc.scalar.activation(hbuf[:, w * F:(w + 1) * F], hP[:],
                                     ACT.Copy, scale=dinvS[:, w:w + 1])
                if w == WEND[0]:
                    emit_shard(0, 0)
                if w == WEND[1]:
                    emit_shard(1, 0)
            poke_bg(0)
            emit_shard(2, 0)

            # ---------------- layers ----------------
            qctr = [0]
            poolP = None
            gtiles = [{}, {}, {}]      # in-flight gather tiles, keyed by window

            def emit_gather(s, w, tab):
                if mm[s][w] == 0:
                    return
                n = 128 * mm[s][w]
                gt = gpools[s].tile([128, Mmax[s] * F], bf16, tag=f"g{s}")
                nc.gpsimd.dma_gather(
                    gt[:, :mm[s][w] * F].rearrange("p (c e) -> p c e", e=F),
                    tab[:],
                    srcS[s][:, offw[s][w] // 16:(offw[s][w] + n) // 16],
                    n, n, F,
                    queue_num=qctr[0] % 4)
                qctr[0] += 1
                gtiles[s][w] = gt

            for l in range(NL):
                tabs = [tables[s][l % 2] for s in range(3)]
                last = l == NL - 1
                if last:
                    poolP = psacc.tile([128, GR], f32, tag="poolP")

                indtiles = {}
                igtiles = {}

                def emit_window_ig(w):
                    t = indp.tile([128, GR], bf16, tag="ig")
                    nc.sync.dma_start(t[:], P["igS"][:, w * GR:(w + 1) * GR])
                    igtiles[w] = t

                def emit_window_ind(w):
                    nsub = mm[0][w] + mm[1][w] + mm[2][w]
                    it = indp.tile([128, MImax * 128], bf16, tag="ind")
                    nc.sync.dma_start(
                        it[:, :nsub * 128],
                        P["ind"][:, offS[w] * 128:(offS[w] + nsub) * 128])
                    indtiles[w] = it

                ewS = [0, 0, 0]
                ei = 0
                for w in range(W):
                    for s in range(3):
                        while ewS[s] <= min(w + cfg.PF, W - 1):
                            emit_gather(s, ewS[s], tabs[s])
                            ewS[s] += 1
                    while ei <= min(w + cfg.PFI, W - 1):
                        emit_window_ind(ei)
                        if last:
                            emit_window_ig(ei)
                        ei += 1

                    nsub = mm[0][w] + mm[1][w] + mm[2][w]
                    it = indtiles.pop(w)
                    gts = [gtiles[s].pop(w) for s in range(3)]
                    SP = ps3.tile([128, F], f32, tag="SP")
                    i = 0
                    for s in range(3):
                        for j in range(mm[s][w]):
                            nc.tensor.matmul(
                                SP[:], it[:, i * 128:(i + 1) * 128],
                                gts[s][:, j * F:(j + 1) * F],
                                start=(i == 0), stop=False)
                            i += 1
                    # self-loop: SP += diag(dinv_w) @ hbuf_w, then relu
                    nc.tensor.matmul(SP[:], diagS[:, w * 128:(w + 1) * 128],
                                     hbuf[:, w * F:(w + 1) * F],
                                     start=False, stop=True)
                    xn = work.tile([128, F], bf16, tag="xn")
                    nc.scalar.activation(xn[:], SP[:], ACT.Relu)

                    if not last:
                        xtP = ps1.tile([128, F], bf16, tag="xtP")
                        nc.tensor.transpose(xtP[:], xn[:], ident[:])
                        xt = work.tile([128, F], bf16, tag="xt")
                        nc.vector.tensor_copy(xt[:], xtP[:])
                        hP = ps.tile([128, F], f32, tag="hP")
                        nc.tensor.matmul(hP[:], xt[:],
                                         wgS[:, (l + 1) * F:(l + 2) * F],
                                         start=True, stop=True)
                        nc.scalar.activation(hbuf[:, w * F:(w + 1) * F], hP[:],
                                             ACT.Copy, scale=dinvS[:, w:w + 1])
                        if w == WEND[0]:
                            emit_shard(0, (l + 1) % 2)
                        if w == WEND[1]:
                            emit_shard(1, (l + 1) % 2)
                        if w == W - 1:
                            poke_bg(l + 1)
                            emit_shard(2, (l + 1) % 2)
                    else:
                        nc.tensor.matmul(poolP[:], xn[:],
                                         igtiles.pop(w)[:],
                                         start=(w == 0), stop=(w == W - 1),
                                         skip_group_check=True)
                assert all(e == W for e in ewS) and ei == W

            # ---------------- pooling allreduce + head ----------------
            sumsS = work.tile([128, GR], f32, tag="sumsS")
            nc.vector.tensor_copy(sumsS[:], poolP[:])
            nc.sync.dma_start(arInD[:], sumsS[:])
            nc.gpsimd.collective_compute(
                "AllReduce", mybir.AluOpType.add, replica_groups=rg,
                ins=[arInD[:]], outs=[arOutD[:]])
            pooledT = work.tile([128, GR], f32, tag="pooledT")
            nc.sync.dma_start(pooledT[:], arOutD[:])

            h1 = []
            for h in range(2):
                h1P = ps3.tile([128, GR], f32, tag="SP")
                nc.tensor.matmul(h1P[:], w1S[:, h * 128:(h + 1) * 128],
                                 pooledT[:], start=True, stop=True)
                h1S = work.tile([128, GR], f32, tag=f"h1S{h}")
                nc.scalar.activation(h1S[:], h1P[:], ACT.Relu,
                                     bias=b1S[:, h:h + 1])
                h1.append(h1S)
            for g in range(GR // 128):
                oP = ps3.tile([128, cfg.LD], f32, tag="SP")
                nc.tensor.matmul(oP[:], h1[0][:, g * 128:(g + 1) * 128],
                                 w2S[:, 0:cfg.LD], start=True, stop=False)
                nc.tensor.matmul(oP[:], h1[1][:, g * 128:(g + 1) * 128],
                                 w2S[:, cfg.LD:2 * cfg.LD], start=False, stop=True)
                oS = work.tile([128, cfg.LD], f32, tag="oS")
                nc.vector.tensor_tensor(oS[:], oP[:], b2B[:], AL.add)
                nc.sync.dma_start(out_ext[g * 128:(g + 1) * 128, :], oS[:])

    nc.compile()
    return nc


# ---------------------------------------------------------------------------
# Entry point
# ---------------------------------------------------------------------------
_CACHE = {}


def _build(cfg, meta):
    key = tuple(tuple(m) for m in meta.m)
    if key not in _CACHE:
        _CACHE[key] = build_graph(cfg, meta)
    return _CACHE[key]


def kernel(**inputs) -> np.ndarray:
    from concourse.bass_utils import run_bass_kernel_spmd
    cfg = Cfg()
    meta = host_prep(cfg, **inputs)
    nc = _build(cfg, meta)
    res = run_bass_kernel_spmd(nc, meta.in_maps, list(range(cfg.C)))
    return np.asarray(res.results[0]["out"], dtype=np.float32)
